# revision 12
# baseline (speedup 1.0000x reference)
"""Distributed GAT forward on 8 Trainium2 NeuronCores (Bass/Tile).

Node-partitioned across 8 cores. Per core, nodes are host-permuted to
[evens, odds] so the transformed-node gather tables split into two
<32768-row halves addressable by int16 dma_gather indices. Dense phases run
feature-major on the TensorEngine; GATConv aggregation gathers transformed
rows per edge (dma_gather) and computes per-destination softmax-weighted
sums as PE matmuls against eq-mask one-hots scaled by exp(leakyrelu(.)).
Tables are exchanged between cores with AllGather collectives; the softmax
denominator rides along as a ones-column of the table.
"""
import sys

sys.path.insert(0, "/opt/trn_rl_repo")

import numpy as np
import ml_dtypes

from concourse import bacc, tile, bass
import concourse.mybir as mybir
from concourse import bass_utils

bf16 = mybir.dt.bfloat16
f32 = mybir.dt.float32
i16 = mybir.dt.int16
nbf = ml_dtypes.bfloat16

N, E, HID, LM, SUB = 50000, 800000, 160, 768, 32
NCORES = 8
NPC = N // NCORES            # 6250
HALF = NPC // 2              # 3125
DT = 64                      # dst-tile size
NT = (NPC + DT - 1) // DT    # 98
TROW = 256                   # bf16 elems per table row (512B)
NB = 512
BLOCKS = [(i * NB, min(NB, NPC - i * NB)) for i in range((NPC + NB - 1) // NB)]
NTILE128 = (NPC + 127) // 128  # 49 node tiles for T-build
PAD_DST = 200.0

AF = mybir.ActivationFunctionType
AL = mybir.AluOpType

_cache = {}


def _build(CP, debug=False):
    C = 2 * CP
    GI = CP * 128
    W16 = CP * 8          # idx columns per parity
    nc = bacc.Bacc("TRN2", target_bir_lowering=False, debug=False,
                   num_devices=NCORES)

    def din(name, shape, dtype):
        return nc.dram_tensor(name, shape, dtype, kind="ExternalInput")

    preT = din("preT", [LM, NPC], bf16)
    desT = din("desT", [LM, NPC], bf16)
    twT = din("twT", [LM, NPC], bf16)
    npT_d = din("npT", [6, NPC], bf16)
    ncT_d = din("ncT", [11, NPC], bf16)
    wnp_d = din("wnp", [6, SUB], bf16)
    wnc_d = din("wnc", [11, SUB], bf16)
    wdes_d = din("wdes", [128, 6 * SUB], bf16)    # chunk-major packed
    wtext_d = din("wtext", [128, 6 * SUB], bf16)
    wtw_d = din("wtw", [128, 6 * SUB], bf16)
    encb96_d = din("encb96", [96, 1], f32)
    encbtv_d = din("encbtv", [SUB, 1], f32)
    encbpv_d = din("encbpv", [SUB, 1], f32)
    win96_d = din("win96", [96, HID], bf16)
    wintv_d = din("wintv", [32, HID], bf16)
    winpv_d = din("winpv", [32, HID], bf16)
    binb_d = din("binb", [128, 1], f32)
    binb2_d = din("binb2", [32, 1], f32)
    w1_a_d = din("w1_a", [128, 162], bf16)
    w1_b_d = din("w1_b", [32, 162], bf16)
    w2_a_d = din("w2_a", [128, 162], bf16)
    w2_b_d = din("w2_b", [32, 162], bf16)
    b1r_d = din("b1r", [128, HID], f32)
    b2r_d = din("b2r", [128, HID], f32)
    wo1_a_d = din("wo1_a", [128, 80], bf16)
    wo1_b_d = din("wo1_b", [32, 80], bf16)
    bo1_d = din("bo1", [80, 1], f32)
    wo2_d = din("wo2", [80, 2], bf16)
    bo2_d = din("bo2", [2, 1], f32)
    idxw_d = din("idxw", [NT, 128, 2 * W16], i16)
    dstl_d = din("dstl", [NT, 128, C], bf16)
    emT_d = nc.dram_tensor("emT", [80, NPC], f32, kind="ExternalOutput")
    outT_d = nc.dram_tensor("outT", [2, NPC], f32, kind="ExternalOutput")
    dbg = {}
    if debug:
        for nm, shp in (("h2a", [128, NPC]), ("h2b", [32, NPC]),
                        ("hwa1", [128, NPC]), ("hwb1", [34, NPC]),
                        ("adB1", [128, NPC]),
                        ("h3a", [128, NPC]), ("h3b", [32, NPC])):
            dbg[nm] = nc.dram_tensor("dbg_" + nm, shp, bf16,
                                     kind="ExternalOutput")

    with tile.TileContext(nc) as tc:
        with (
            tc.tile_pool(name="persist", bufs=1) as pers,
            tc.tile_pool(name="xt", bufs=4) as xt,
            tc.tile_pool(name="work", bufs=2) as wk,
            tc.tile_pool(name="gat", bufs=2) as gp,
            tc.tile_pool(name="psA", bufs=2, space="PSUM") as psA,
            tc.tile_pool(name="psB", bufs=2, space="PSUM") as psB,
            tc.tile_pool(name="psG", bufs=2, space="PSUM") as psG,
            tc.tile_pool(name="psT", bufs=2, space="PSUM") as psT,
            tc.tile_pool(name="dram", bufs=1, space="DRAM") as dram,
        ):
            # ---------------- constants ----------------
            iota_i = pers.tile([128, 128], i16)
            nc.gpsimd.iota(iota_i[:], pattern=[[1, 128]], base=0,
                           channel_multiplier=0)
            iota_b = pers.tile([128, 128], bf16)
            nc.vector.tensor_copy(iota_b[:], iota_i[:])
            iota_p = pers.tile([128, 1], i16)
            nc.gpsimd.iota(iota_p[:], pattern=[[0, 1]], base=0,
                           channel_multiplier=1)
            iota_pb = pers.tile([128, 1], bf16)
            nc.vector.tensor_copy(iota_pb[:], iota_p[:])
            ident = pers.tile([128, 128], bf16)
            nc.vector.tensor_tensor(ident[:], iota_b[:],
                                    iota_pb[:].broadcast_to([128, 128]),
                                    AL.is_equal)
            identf = pers.tile([128, 128], f32)
            nc.vector.tensor_copy(identf[:], ident[:])
            ones_row = pers.tile([1, 128], bf16)
            nc.vector.memset(ones_row[:], 1.0)
            ones_col = pers.tile([128, 1], bf16)
            nc.vector.memset(ones_col[:], 1.0)

            # ---------------- persistent state ----------------
            ADW = NT * DT  # 6272, padded for 64-wide slicing
            h2a = pers.tile([128, NPC], bf16)
            h2b = pers.tile([32, NPC], bf16)
            h3a = pers.tile([128, NPC], bf16)
            h3b = pers.tile([32, NPC], bf16)
            hwa = pers.tile([128, NPC], bf16)
            hwb = pers.tile([34, NPC], bf16)
            adB = pers.tile([128, ADW], bf16)
            nc.vector.memset(adB[:, NPC:ADW], 0.0)
            ad_row = pers.tile([1, NPC], bf16)

            # ---------------- weights ----------------
            def wload(d, shape, dtype=bf16):
                t = pers.tile(shape, dtype, tag="w_" + d.name)
                nc.sync.dma_start(t[:], d[:])
                return t

            wnp_t = wload(wnp_d, [6, SUB])
            wnc_t = wload(wnc_d, [11, SUB])
            wdes_t = wload(wdes_d, [128, 6 * SUB])
            wtext_t = wload(wtext_d, [128, 6 * SUB])
            wtw_t = wload(wtw_d, [128, 6 * SUB])
            encb96_t = wload(encb96_d, [96, 1], f32)
            encbtv_t = wload(encbtv_d, [SUB, 1], f32)
            encbpv_t = wload(encbpv_d, [SUB, 1], f32)
            win96_t = wload(win96_d, [96, HID])
            wintv_t = wload(wintv_d, [32, HID])
            winpv_t = wload(winpv_d, [32, HID])
            binb_t = wload(binb_d, [128, 1], f32)
            binb2_t = wload(binb2_d, [32, 1], f32)
            w1_a_t = wload(w1_a_d, [128, 162])
            w1_b_t = wload(w1_b_d, [32, 162])
            w2_a_t = wload(w2_a_d, [128, 162])
            w2_b_t = wload(w2_b_d, [32, 162])
            b1r_t = wload(b1r_d, [128, HID], f32)
            b2r_t = wload(b2r_d, [128, HID], f32)
            wo1_a_t = wload(wo1_a_d, [128, 80])
            wo1_b_t = wload(wo1_b_d, [32, 80])
            bo1_t = wload(bo1_d, [80, 1], f32)
            wo2_t = wload(wo2_d, [80, 2])
            bo2_t = wload(bo2_d, [2, 1], f32)

            def lrelu_from(out_ap, in_ap, alpha):
                # out = max(alpha*in, in)
                shp = [in_ap.shape[0], int(np.prod(in_ap.shape[1:]))]
                tmp = wk.tile([128, NB], f32, tag="lrtmp")
                ta = tmp[0:shp[0], 0:shp[1]]
                nc.vector.tensor_scalar_mul(ta, in_ap, float(alpha))
                nc.vector.tensor_tensor(out_ap, in_ap, ta, AL.max)

            # ---------------- encoder ----------------
            for (b0, nb) in BLOCKS:
                pe = psA.tile([128, NB], f32, tag="psA")
                pe_tv = psB.tile([SUB, NB], f32, tag="psB")
                pe_pv = psB.tile([SUB, NB], f32, tag="psB")
                # small encoders -> pe rows 0:32 (nv), 32:64 (cv)
                xn = xt.tile([6, NB], bf16, tag="xt6")
                nc.sync.dma_start(xn[:, 0:nb], npT_d[:, b0:b0 + nb])
                nc.tensor.matmul(pe[0:SUB, 0:nb], wnp_t[:], xn[:, 0:nb],
                                 start=True, stop=True)
                xc = xt.tile([11, NB], bf16, tag="xt11")
                nc.sync.dma_start(xc[:, 0:nb], ncT_d[:, b0:b0 + nb])
                nc.tensor.matmul(pe[SUB:2 * SUB, 0:nb], wnc_t[:], xc[:, 0:nb],
                                 start=True, stop=True)
                # big encoders: dv -> pe rows 64:96; tv -> pe2 rows 0:32;
                # pv -> pe2 rows 32:64
                for (src_d, w_t, prow, ptile) in (
                    (desT, wdes_t, 2 * SUB, pe),
                    (twT, wtext_t, 0, pe_tv),
                    (preT, wtw_t, 0, pe_pv),
                ):
                    for k in range(6):
                        xb = xt.tile([128, NB], bf16, tag="xtb")
                        nc.sync.dma_start(xb[:, 0:nb],
                                          src_d[128 * k:128 * (k + 1),
                                                b0:b0 + nb])
                        nc.tensor.matmul(
                            ptile[prow:prow + SUB, 0:nb],
                            w_t[:, SUB * k:SUB * (k + 1)], xb[:, 0:nb],
                            start=(k == 0), stop=(k == 5))
                y1 = wk.tile([96, NB], f32, tag="y1")
                nc.scalar.activation(y1[:, 0:nb], pe[0:96, 0:nb], AF.Identity,
                                     bias=encb96_t[:])
                h1abc = wk.tile([96, NB], bf16, tag="h1abc")
                lrelu_from(h1abc[:, 0:nb], y1[:, 0:nb], 0.01)
                y2 = wk.tile([SUB, NB], f32, tag="y2")
                nc.scalar.activation(y2[:, 0:nb], pe_tv[:, 0:nb], AF.Identity,
                                     bias=encbtv_t[:])
                h1tv = wk.tile([SUB, NB], bf16, tag="h1tv")
                lrelu_from(h1tv[:, 0:nb], y2[:, 0:nb], 0.01)
                y3 = wk.tile([SUB, NB], f32, tag="y3")
                nc.scalar.activation(y3[:, 0:nb], pe_pv[:, 0:nb], AF.Identity,
                                     bias=encbpv_t[:])
                h1pv = wk.tile([SUB, NB], bf16, tag="h1pv")
                lrelu_from(h1pv[:, 0:nb], y3[:, 0:nb], 0.01)
                # w_in over K-chunks [96 | 32(tv) | 32(pv)]
                ph = psA.tile([128, NB], f32, tag="psA")
                nc.tensor.matmul(ph[:, 0:nb], win96_t[:, 0:128],
                                 h1abc[:, 0:nb], start=True, stop=False)
                nc.tensor.matmul(ph[:, 0:nb], wintv_t[:, 0:128],
                                 h1tv[:, 0:nb], start=False, stop=False)
                nc.tensor.matmul(ph[:, 0:nb], winpv_t[:, 0:128],
                                 h1pv[:, 0:nb], start=False, stop=True)
                ph2 = psB.tile([32, NB], f32, tag="psB")
                nc.tensor.matmul(ph2[:, 0:nb], win96_t[:, 128:160],
                                 h1abc[:, 0:nb], start=True, stop=False)
                nc.tensor.matmul(ph2[:, 0:nb], wintv_t[:, 128:160],
                                 h1tv[:, 0:nb], start=False, stop=False)
                nc.tensor.matmul(ph2[:, 0:nb], winpv_t[:, 128:160],
                                 h1pv[:, 0:nb], start=False, stop=True)
                ya = wk.tile([128, NB], f32, tag="ya")
                nc.scalar.activation(ya[:, 0:nb], ph[:, 0:nb], AF.Identity,
                                     bias=binb_t[:])
                lrelu_from(h2a[:, b0:b0 + nb], ya[:, 0:nb], 0.01)
                yb = wk.tile([32, NB], f32, tag="yb")
                nc.scalar.activation(yb[:, 0:nb], ph2[:, 0:nb], AF.Identity,
                                     bias=binb2_t[:])
                lrelu_from(h2b[:, b0:b0 + nb], yb[:, 0:nb], 0.01)

            # ---------------- convs ----------------
            def conv(k, ha, hb, wa_t, wb_t, brep_t, oa, ob):
                # transform: hW^T (+ a_s, a_d rows) feature-major
                for (b0, nb) in BLOCKS:
                    pw = psA.tile([128, NB], f32, tag="psA")
                    nc.tensor.matmul(pw[:, 0:nb], wa_t[:, 0:128],
                                     ha[:, b0:b0 + nb], start=True, stop=False)
                    nc.tensor.matmul(pw[:, 0:nb], wb_t[:, 0:128],
                                     hb[:, b0:b0 + nb], start=False, stop=True)
                    nc.vector.tensor_copy(hwa[:, b0:b0 + nb], pw[:, 0:nb])
                    pw2 = psB.tile([34, NB], f32, tag="psB")
                    nc.tensor.matmul(pw2[:, 0:nb], wa_t[:, 128:162],
                                     ha[:, b0:b0 + nb], start=True, stop=False)
                    nc.tensor.matmul(pw2[:, 0:nb], wb_t[:, 128:162],
                                     hb[:, b0:b0 + nb], start=False, stop=True)
                    nc.vector.tensor_copy(hwb[:, b0:b0 + nb], pw2[:, 0:nb])
                    pw4 = psB.tile([1, NB], f32, tag="psB")
                    nc.tensor.matmul(pw4[:, 0:nb], wa_t[:, 161:162],
                                     ha[:, b0:b0 + nb], start=True, stop=False)
                    nc.tensor.matmul(pw4[:, 0:nb], wb_t[:, 161:162],
                                     hb[:, b0:b0 + nb], start=False, stop=True)
                    nc.vector.tensor_copy(ad_row[:, b0:b0 + nb], pw4[:, 0:nb])
                # replicate a_d (row 33 of hwb) to all 128 partitions
                for (b0, nb) in BLOCKS:
                    pr = psA.tile([128, NB], f32, tag="psA")
                    nc.tensor.matmul(pr[:, 0:nb], ones_row[:],
                                     ad_row[:, b0:b0 + nb],
                                     start=True, stop=True)
                    nc.vector.tensor_copy(adB[:, b0:b0 + nb], pr[:, 0:nb])
                if debug and k == 1:
                    nc.sync.dma_start(dbg["hwa1"][:], hwa[:])
                    nc.sync.dma_start(dbg["hwb1"][:], hwb[:])
                    nc.sync.dma_start(dbg["adB1"][:], adB[:, 0:NPC])
                # build node-major tables, scatter to parity shards
                shE = dram.tile([HALF, TROW], bf16, tag=f"shE{k}")
                shO = dram.tile([HALF, TROW], bf16, tag=f"shO{k}")
                for j in range(NTILE128):
                    base = 128 * j
                    cnt = min(128, NPC - base)
                    p1 = psT.tile([128, 128], bf16, tag="pst")
                    nc.tensor.transpose(p1[0:cnt, :],
                                        hwa[:, base:base + cnt], ident[:])
                    p2 = psT.tile([128, 34], bf16, tag="pst")
                    nc.tensor.transpose(p2[0:cnt, :],
                                        hwb[0:34, base:base + cnt],
                                        ident[0:34, 0:34])
                    trow = wk.tile([128, TROW], bf16, tag="trow")
                    nc.vector.tensor_copy(trow[0:cnt, 0:128], p1[0:cnt, :])
                    nc.vector.tensor_copy(trow[0:cnt, 128:160],
                                          p2[0:cnt, 0:32])
                    nc.vector.tensor_copy(trow[0:cnt, 160:161],
                                          ones_col[0:cnt, :])
                    nc.vector.tensor_copy(trow[0:cnt, 161:162],
                                          p2[0:cnt, 32:33])
                    e_end = min(max(HALF - base, 0), cnt)
                    if e_end > 0:
                        nc.sync.dma_start(shE[base:base + e_end, :],
                                          trow[0:e_end, 0:TROW])
                    if e_end < cnt:
                        o0 = base + e_end - HALF
                        nc.sync.dma_start(shO[o0:o0 + cnt - e_end, :],
                                          trow[e_end:cnt, 0:TROW])
                # exchange
                TE = dram.tile([NCORES * HALF, TROW], bf16, tag=f"TE{k}")
                TO = dram.tile([NCORES * HALF, TROW], bf16, tag=f"TO{k}")
                nc.gpsimd.collective_compute(
                    "AllGather", AL.bypass,
                    replica_groups=[list(range(NCORES))],
                    ins=[shE[:].opt()], outs=[TE[:].opt()])
                nc.gpsimd.collective_compute(
                    "AllGather", AL.bypass,
                    replica_groups=[list(range(NCORES))],
                    ins=[shO[:].opt()], outs=[TO[:].opt()])
                # aggregation per dst tile
                idxd = idxw_d
                dstd = dstl_d
                for t in range(NT):
                    d0 = t * DT
                    dcnt = min(DT, NPC - d0)
                    idx_t = gp.tile([128, 2 * W16], i16, tag="idx")
                    nc.sync.dma_start(idx_t[:], idxd[t])
                    dst_t = gp.tile([128, C], bf16, tag="dstl")
                    nc.sync.dma_start(dst_t[:], dstd[t])
                    g = gp.tile([128, C, TROW], bf16, tag="g")
                    nc.gpsimd.dma_gather(
                        out_ap=g[:, 0:CP, :], in_ap=TE[:],
                        idxs_ap=idx_t[:, 0:W16], num_idxs=GI,
                        num_idxs_reg=GI, elem_size=TROW)
                    nc.gpsimd.dma_gather(
                        out_ap=g[:, CP:C, :], in_ap=TO[:],
                        idxs_ap=idx_t[:, W16:2 * W16], num_idxs=GI,
                        num_idxs_reg=GI, elem_size=TROW)
                    s0 = gp.tile([128, C, DT], bf16, tag="s0")
                    nc.vector.tensor_tensor(
                        s0[:],
                        iota_b[:, 0:DT].unsqueeze(1).broadcast_to([128, C, DT]),
                        dst_t[:].unsqueeze(2).broadcast_to([128, C, DT]),
                        AL.is_equal)
                    pp = gp.tile([128, C, DT], bf16, tag="pp")
                    nc.vector.tensor_tensor(
                        pp[:], s0[:],
                        adB[:, d0:d0 + DT].unsqueeze(1)
                           .broadcast_to([128, C, DT]),
                        AL.mult)
                    x1 = wk.tile([128, C], f32, tag="x1")
                    nc.vector.tensor_reduce(x1[:], pp[:],
                                            mybir.AxisListType.X, AL.add)
                    x2 = wk.tile([128, C], f32, tag="x2")
                    nc.vector.tensor_tensor(x2[:], x1[:], g[:, :, 161],
                                            AL.add)
                    e1 = wk.tile([128, C], f32, tag="e1")
                    lrelu_from(e1[:], x2[:], 0.2)
                    e2 = wk.tile([128, C], bf16, tag="e2")
                    nc.scalar.activation(e2[:], e1[:], AF.Exp)
                    s1 = gp.tile([128, C, DT], bf16, tag="s1")
                    nc.vector.tensor_tensor(
                        s1[:], s0[:],
                        e2[:].unsqueeze(2).broadcast_to([128, C, DT]),
                        AL.mult)
                    pg = psG.tile([DT, 161], f32, tag="pg")
                    for cc in range(C):
                        nc.tensor.matmul(pg[:], s1[:, cc, :],
                                         g[:, cc, 0:161],
                                         start=(cc == 0), stop=(cc == C - 1))
                    zadj = wk.tile([DT, 1], f32, tag="zadj")
                    nc.vector.tensor_scalar_add(zadj[:], pg[:, 160:161], 1e-16)
                    zrec = wk.tile([DT, 1], f32, tag="zrec")
                    nc.vector.reciprocal(zrec[:], zadj[:])
                    o = wk.tile([DT, HID], f32, tag="o")
                    nc.vector.scalar_tensor_tensor(
                        o[:], pg[:, 0:160], zrec[:], brep_t[0:DT, :],
                        AL.mult, AL.add)
                    px = psT.tile([128, DT], f32, tag="pst")
                    nc.tensor.transpose(px[:, 0:dcnt], o[0:dcnt, 0:128],
                                        identf[0:dcnt, 0:dcnt])
                    nc.vector.tensor_copy(oa[:, d0:d0 + dcnt], px[:, 0:dcnt])
                    px2 = psT.tile([32, DT], f32, tag="pst")
                    nc.tensor.transpose(px2[:, 0:dcnt], o[0:dcnt, 128:160],
                                        identf[0:dcnt, 0:dcnt])
                    nc.vector.tensor_copy(ob[:, d0:d0 + dcnt],
                                          px2[:, 0:dcnt])

            if debug:
                nc.sync.dma_start(dbg["h2a"][:], h2a[:])
                nc.sync.dma_start(dbg["h2b"][:], h2b[:])
            conv(1, h2a, h2b, w1_a_t, w1_b_t, b1r_t, h3a, h3b)
            if debug:
                nc.sync.dma_start(dbg["h3a"][:], h3a[:])
                nc.sync.dma_start(dbg["h3b"][:], h3b[:])
            conv(2, h3a, h3b, w2_a_t, w2_b_t, b2r_t, h2a, h2b)

            # ---------------- output layers ----------------
            for (b0, nb) in BLOCKS:
                pm = psA.tile([80, NB], f32, tag="psA")
                nc.tensor.matmul(pm[:, 0:nb], wo1_a_t[:], h2a[:, b0:b0 + nb],
                                 start=True, stop=False)
                nc.tensor.matmul(pm[:, 0:nb], wo1_b_t[:], h2b[:, b0:b0 + nb],
                                 start=False, stop=True)
                ym = wk.tile([80, NB], f32, tag="ym")
                nc.scalar.activation(ym[:, 0:nb], pm[:, 0:nb], AF.Identity,
                                     bias=bo1_t[:])
                emf = wk.tile([80, NB], f32, tag="emf")
                lrelu_from(emf[:, 0:nb], ym[:, 0:nb], 0.01)
                nc.sync.dma_start(emT_d[:, b0:b0 + nb], emf[:, 0:nb])
                emb = wk.tile([80, NB], bf16, tag="emb")
                nc.vector.tensor_copy(emb[:, 0:nb], emf[:, 0:nb])
                po = psB.tile([2, NB], f32, tag="psB")
                nc.tensor.matmul(po[:, 0:nb], wo2_t[:], emb[:, 0:nb],
                                 start=True, stop=True)
                of = wk.tile([2, NB], f32, tag="of")
                nc.scalar.activation(of[:, 0:nb], po[:, 0:nb], AF.Identity,
                                     bias=bo2_t[:])
                nc.sync.dma_start(outT_d[:, b0:b0 + nb], of[:, 0:nb])

    nc.compile()
    return nc


def _wrap_idx(flat):
    """[n] -> [128, n//16] wrapped layout (i at partition i%16, col i//16)."""
    w = flat.reshape(-1, 16).T.astype(np.int16)
    return np.tile(w, (8, 1))


def _prep(inputs):
    """Host-side sharding/permutation/edge prep. Returns (CP, in_maps, perm)."""
    f = {k: np.asarray(v, np.float32) for k, v in inputs.items()
         if np.asarray(v).dtype.kind == "f"}

    # permutation: per core, evens then odds (local indices)
    perm = np.empty(N, np.int64)  # perm[new_global] = old_global
    loc = np.arange(NPC)
    pl = np.empty(NPC, np.int64)
    pl[loc % 2 == 0] = loc[loc % 2 == 0] // 2
    pl[loc % 2 == 1] = HALF + loc[loc % 2 == 1] // 2
    inv_pl = np.argsort(pl)  # inv_pl[new_local] = old_local
    for c in range(NCORES):
        perm[c * NPC: (c + 1) * NPC] = c * NPC + inv_pl

    def shardT(x, dtype=nbf):
        # x [N, F] -> per-core [F, NPC] column-permuted
        xp = x[perm]  # rows in new order
        return [np.ascontiguousarray(xp[c * NPC:(c + 1) * NPC].T).astype(dtype)
                for c in range(NCORES)]

    pre_s = shardT(f["pre_x"])
    des_s = shardT(f["des_tensor"])
    tw_s = shardT(f["tweet_tensor"])
    np_s = shardT(f["num_prop"])
    nc_s = shardT(f["num_category"])

    def pack768(w):  # [32, 768] -> [128, 192] chunk-major of w.T
        wt = np.ascontiguousarray(w.T)  # [768, 32]
        return np.ascontiguousarray(
            wt.reshape(6, 128, SUB).transpose(1, 0, 2).reshape(128, 6 * SUB)
        ).astype(nbf)

    wb = {
        "wnp": np.ascontiguousarray(f["w_np"].T).astype(nbf),
        "wnc": np.ascontiguousarray(f["w_nc"].T).astype(nbf),
        "wdes": pack768(f["w_des"]),
        "wtext": pack768(f["w_text"]),
        "wtw": pack768(f["w_tw"]),
        "encb96": np.concatenate([f["b_np"], f["b_nc"],
                                  f["b_des"]]).reshape(96, 1)
                    .astype(np.float32),
        "encbtv": f["b_text"].reshape(SUB, 1).astype(np.float32),
        "encbpv": f["b_tw"].reshape(SUB, 1).astype(np.float32),
        "binb": f["b_in"][0:128].reshape(128, 1).astype(np.float32),
        "binb2": f["b_in"][128:160].reshape(32, 1).astype(np.float32),
        "b1r": np.tile(f["b1"][None, :], (128, 1)).astype(np.float32),
        "b2r": np.tile(f["b2"][None, :], (128, 1)).astype(np.float32),
        "bo1": f["b_o1"].reshape(80, 1).astype(np.float32),
        "bo2": f["b_o2"].reshape(2, 1).astype(np.float32),
        "wo2": np.ascontiguousarray(f["w_o2"].T).astype(nbf),
    }
    winT = np.ascontiguousarray(f["w_in"].T).astype(nbf)       # [160,160]
    wb["win96"] = np.ascontiguousarray(winT[0:96])
    wb["wintv"] = np.ascontiguousarray(winT[96:128])
    wb["winpv"] = np.ascontiguousarray(winT[128:160])
    wo1T = np.ascontiguousarray(f["w_o1"].T).astype(nbf)       # [160,80]
    wb["wo1_a"], wb["wo1_b"] = wo1T[0:128], wo1T[128:160]
    for k, W, As, Ad in (("w1", f["W1"], f["as1"], f["ad1"]),
                         ("w2", f["W2"], f["as2"], f["ad2"])):
        aug = np.concatenate([W.T, (W.T @ As)[:, None],
                              (W.T @ Ad)[:, None]], 1)  # [160,162]
        aug = np.ascontiguousarray(aug).astype(nbf)
        wb[k + "_a"], wb[k + "_b"] = aug[0:128], aug[128:160]

    # ---- edges ----
    ei = np.asarray(inputs["edge_index"]).astype(np.int64)
    loops = np.arange(N, dtype=np.int64)
    src = np.concatenate([ei[0], loops])
    dst = np.concatenate([ei[1], loops])
    core = dst // NPC
    dl_old = dst % NPC
    dl_new = np.where(dl_old % 2 == 0, dl_old // 2, HALF + dl_old // 2)
    tile_id = dl_new // DT
    dloc = (dl_new % DT).astype(np.float32)
    par = (src % 2).astype(np.int64)
    srow = (src >> 1).astype(np.int16)

    key = (core * NT + tile_id) * 2 + par
    order = np.argsort(key, kind="stable")
    key_s = key[order]
    srow_s = srow[order]
    dloc_s = dloc[order]
    nbuck = NCORES * NT * 2
    counts = np.bincount(key_s, minlength=nbuck)
    starts = np.concatenate([[0], np.cumsum(counts)[:-1]])
    CP = int((counts.max() + 127) // 128)
    GI = CP * 128
    C = 2 * CP

    pos_in_bucket = np.arange(len(key_s)) - starts[key_s]

    # gather index arrays [ncore, NT, 2, GI], pad with row 0
    idx_full = np.zeros((NCORES, NT, 2, GI), np.int16)
    dst_full = np.full((NCORES, NT, 128, C), PAD_DST, np.float32)
    b_core = key_s // (NT * 2)
    b_tile = (key_s // 2) % NT
    b_par = key_s % 2
    idx_full[b_core, b_tile, b_par, pos_in_bucket] = srow_s
    chunk = b_par * CP + pos_in_bucket // 128
    partn = pos_in_bucket % 128
    dst_full[b_core, b_tile, partn, chunk] = dloc_s

    in_maps = []
    for c in range(NCORES):
        m = dict(wb)
        m["preT"], m["desT"], m["twT"] = pre_s[c], des_s[c], tw_s[c]
        m["npT"], m["ncT"] = np_s[c], nc_s[c]
        iw = np.zeros((NT, 128, 2 * CP * 8), np.int16)
        for t in range(NT):
            iw[t, :, 0:CP * 8] = _wrap_idx(idx_full[c, t, 0])
            iw[t, :, CP * 8:] = _wrap_idx(idx_full[c, t, 1])
        dstl = dst_full[c].astype(nbf)
        m["idxw"] = iw
        m["dstl"] = dstl
        in_maps.append(m)
    return CP, in_maps, perm


def kernel(**inputs):
    CP, in_maps, perm = _prep(inputs)
    if CP not in _cache:
        _cache[CP] = _build(CP)
    nc = _cache[CP]
    res = None
    last = None
    for attempt in range(4):
        try:
            res = bass_utils.run_bass_kernel_spmd(
                nc, in_maps, core_ids=list(range(NCORES)))
            break
        except Exception as e:  # transient NRT_EXEC_UNIT_UNRECOVERABLE
            last = e
            import time as _time
            _time.sleep(5)
    if res is None:
        raise last
    em = np.empty((N, 80), np.float32)
    out = np.empty((N, 2), np.float32)
    for c in range(NCORES):
        r = res.results[c]
        em[perm[c * NPC:(c + 1) * NPC]] = r["emT"].T
        out[perm[c * NPC:(c + 1) * NPC]] = r["outT"].T
    return (out, em)
